# revision 21
# baseline (speedup 1.0000x reference)
"""GAT attention head (gnn_message_passing) on 8 TRN2 NeuronCores.

Strategy v5 (dst-sharded, gather-free recompute):
  - Per-edge h' rows are RECOMPUTED on device instead of gathered: the host
    ships X re-ordered per edge slot (X_edge, contiguous streaming reads),
    and each 128-slot chunk does two K=128 matmuls against the extended
    weight wext [256, 129] = [W | W@a_src], yielding ps = [h' (0:128) |
    e_src_raw (128)] in PSUM. This avoids all per-edge DMA descriptors
    (SWDGE desc-gen at ~8.5ns/desc and random 512-B HBM reads were the
    bottleneck of gather-based variants).
  - Slot structure: edges sharded by dst core, each dst node (w, r) owns
    partition r of R chunk-columns of its 128-dst window (identity one-hot
    => no matmul for aggregation): chunk col c = w*R + j. Per chunk, one
    fused DVE op accumulates acc[:, 0:128] += fm * ps[:, 0:128] (PSUM read,
    in-place SBUF accumulate). fm = exp(leakyrelu(e_dst + e_src + b)) with
    e_dst a per-window broadcast of the on-chip edcols column, pad masks
    (-30000) and b_src+b_dst folded into a host mask slab, and the softmax
    denominator taken from the EXP's accum_out (no ones column).
  - Leftover edges (deg > R) go to a generic overflow stream: recompute ps,
    evacuate rows to SBUF, full [128, WIN] score matrix against a PE
    broadcast of the window's e_dst row, selw = onehot*fm, and two matmuls
    (rows + ones-den) into a [128, 129] PSUM window accumulator.
  - e_dst per node comes from 2 tiny matmuls per 128-node tile against
    wd = W@a_dst (phase 1); no AllGather / collectives at all.
  - Final pass per window: out = elu(num/max(den,1e-12) + output_bias).
Output: each core writes its 6250-row slab; host concatenates.
"""

import os
import sys

for _p in ("/opt/trn_rl_repo", "/root/.axon_site/_ro/trn_rl_repo"):
    if os.path.isdir(_p) and _p not in sys.path:
        sys.path.append(_p)

import numpy as np
import ml_dtypes

import concourse.bass as bass
import concourse.mybir as mybir
import concourse.tile as tile
from concourse import bacc
from concourse.bass_utils import run_bass_kernel_spmd

NC_ = 8
N = 50000
E = 800000
IN_DIM = 256
OUT_DIM = 128
NSH = N // NC_           # 6250 nodes per core
WIN = 128                # dst window size
NWIN = (NSH + WIN - 1) // WIN   # 49
PW = 129                 # ps width: h'(128) + e_src_raw col
R = int(os.environ.get("KERNEL_R", "16"))     # slots per dst node
KB = int(os.environ.get("KERNEL_KB", "6"))    # chunks per X block
F32 = mybir.dt.float32
BF16 = mybir.dt.bfloat16

LAST_EXEC_NS = None

_GRAPH_CACHE = {}


def _prep_edges(edge_src, edge_dst):
    """Partition edges by dst core, build the fixed R-slot main grid plus a
    generic overflow stream, padded to chunk counts shared by all cores."""
    edge_src = np.asarray(edge_src).astype(np.int64)
    edge_dst = np.asarray(edge_dst).astype(np.int64)
    core = edge_dst // NSH
    grids = []
    ovfs = []
    OVC = np.zeros(NWIN, np.int64)
    for k in range(NC_):
        m = core == k
        s = edge_src[m]
        d = edge_dst[m] - k * NSH
        order = np.argsort(d, kind="stable")
        s, d = s[order], d[order]
        g = np.full((NSH, R), -1, np.int64)
        ov = [[] for _ in range(NWIN)]
        fill = np.zeros(NSH, np.int32)
        for i in range(len(s)):
            dd = d[i]
            f = fill[dd]
            if f < R:
                g[dd, f] = s[i]
                fill[dd] = f + 1
            else:
                ov[dd // WIN].append((s[i], dd - (dd // WIN) * WIN))
        grids.append(g)
        ovfs.append(ov)
        OVC = np.maximum(OVC, [(len(v) + 127) // 128 for v in ov])
    Cov = int(OVC.sum())
    ovoffs = np.zeros(NWIN + 1, np.int64)
    ovoffs[1:] = np.cumsum(OVC) * 128
    Cm = NWIN * R
    maps = []
    for k in range(NC_):
        g = grids[k]
        gfull = np.full((NWIN * WIN, R), -1, np.int64)
        gfull[:NSH] = g
        arr = gfull.reshape(NWIN, WIN, R).transpose(0, 2, 1)  # [w, j, p]
        mlin = arr.reshape(-1)                 # i = c*128 + p
        mmask = np.where(mlin >= 0, 0.0, -30000.0).astype(np.float32)
        mlin = np.where(mlin >= 0, mlin, 0)

        ov = ovfs[k]
        olin = np.zeros(max(Cov, 1) * 128, np.int64)
        odst = np.zeros(max(Cov, 1) * 128, np.float32)
        omask = np.full(max(Cov, 1) * 128, -30000.0, np.float32)
        for wv in range(NWIN):
            o = ovoffs[wv]
            for i, (src, dr) in enumerate(ov[wv]):
                olin[o + i] = src
                odst[o + i] = dr
                omask[o + i] = 0.0
        Cx = max(Cov, 1)
        maps.append({
            "mlin": mlin, "odstl": odst, "olin": olin,
            "mmask": np.ascontiguousarray(mmask.reshape(Cm, 128).T),
            "odst": np.ascontiguousarray(odst.reshape(Cx, 128).T),
            "omask": np.ascontiguousarray(omask.reshape(Cx, 128).T),
        })
    return tuple(OVC.tolist()), Cov, maps


def _build(OVC, Cov):
    Cm = NWIN * R
    nc = bacc.Bacc("TRN2", target_bir_lowering=False, debug=False,
                   enable_asserts=True, num_devices=NC_)
    xT = nc.dram_tensor("xT", [IN_DIM, NSH], BF16, kind="ExternalInput").ap()
    # wexts: [128, 129] each half: [W rows | (W@a_src) col]
    wextA = nc.dram_tensor("wextA", [128, PW], BF16, kind="ExternalInput").ap()
    wextB = nc.dram_tensor("wextB", [128, PW], BF16, kind="ExternalInput").ap()
    wdA = nc.dram_tensor("wdA", [128, 1], BF16, kind="ExternalInput").ap()
    wdB = nc.dram_tensor("wdB", [128, 1], BF16, kind="ExternalInput").ap()
    iota = nc.dram_tensor("iota", [128, 128], BF16, kind="ExternalInput").ap()
    ones_r = nc.dram_tensor("ones_r", [1, 128], BF16, kind="ExternalInput").ap()
    ones_c = nc.dram_tensor("ones_c", [128, 1], BF16, kind="ExternalInput").ap()
    obias = nc.dram_tensor("obias", [128, 128], F32, kind="ExternalInput").ap()
    xmA = nc.dram_tensor("xmA", [128, Cm * 128], BF16, kind="ExternalInput").ap()
    xmB = nc.dram_tensor("xmB", [128, Cm * 128], BF16, kind="ExternalInput").ap()
    mmask = nc.dram_tensor("mmask", [128, Cm], F32, kind="ExternalInput").ap()
    Cx = max(Cov, 1)
    xoA = nc.dram_tensor("xoA", [128, Cx * 128], BF16, kind="ExternalInput").ap()
    xoB = nc.dram_tensor("xoB", [128, Cx * 128], BF16, kind="ExternalInput").ap()
    odst = nc.dram_tensor("odst", [128, Cx], F32, kind="ExternalInput").ap()
    omask = nc.dram_tensor("omask", [128, Cx], F32, kind="ExternalInput").ap()
    out = nc.dram_tensor("out", [NSH, OUT_DIM], F32, kind="ExternalOutput").ap()

    edloc = nc.dram_tensor("edloc", [NWIN * WIN, 1], F32)

    EXP = mybir.ActivationFunctionType.Exp
    AO = mybir.AluOpType

    with tile.TileContext(nc) as tc:
        with tc.tile_pool(name="const", bufs=1) as constp, \
             tc.tile_pool(name="idx", bufs=1) as idxp:
            wA_t = constp.tile([128, PW], BF16)
            nc.sync.dma_start(wA_t[:], wextA[:, :])
            wB_t = constp.tile([128, PW], BF16)
            nc.sync.dma_start(wB_t[:], wextB[:, :])
            wdA_t = constp.tile([128, 1], BF16)
            nc.sync.dma_start(wdA_t[:], wdA[:, :])
            wdB_t = constp.tile([128, 1], BF16)
            nc.sync.dma_start(wdB_t[:], wdB[:, :])
            iota_t = constp.tile([128, 128], BF16)
            nc.sync.dma_start(iota_t[:], iota[:, :])
            ones_t = constp.tile([1, 128], BF16)
            nc.sync.dma_start(ones_t[:], ones_r[:, :])
            onesc_t = constp.tile([128, 1], BF16)
            nc.sync.dma_start(onesc_t[:], ones_c[:, :])
            obias_t = constp.tile([128, 128], F32)
            nc.sync.dma_start(obias_t[:], obias[:, :])
            edcols = constp.tile([128, NWIN], F32)
            mmask_t = idxp.tile([128, Cm], F32)
            nc.sync.dma_start(mmask_t[:], mmask[:, :])
            odst_t = idxp.tile([128, Cx], F32)
            nc.sync.dma_start(odst_t[:], odst[:, :])
            omask_t = idxp.tile([128, Cx], F32)
            nc.sync.dma_start(omask_t[:], omask[:, :])

            # ---- phase 1: per-node e_dst (edcols + edloc) ----
            with tc.tile_pool(name="p1x", bufs=1) as p1x, \
                 tc.tile_pool(name="ps1", bufs=4, space="PSUM") as ps1:
                xt = p1x.tile([128, 2 * NSH], BF16)
                nc.sync.dma_start(xt[:, 0:NSH], xT[0:128, :])
                nc.sync.dma_start(xt[:, NSH:2 * NSH], xT[128:256, :])
                nc.vector.memset(edcols[:], 0.0)
                for m in range(NWIN):
                    pm = min(128, NSH - m * 128)
                    pse = ps1.tile([128, 1], F32, tag="pse")
                    nc.tensor.matmul(out=pse[:pm, :],
                                     lhsT=xt[:, m * 128: m * 128 + pm],
                                     rhs=wdA_t[:], start=True, stop=False)
                    nc.tensor.matmul(out=pse[:pm, :],
                                     lhsT=xt[:, NSH + m * 128: NSH + m * 128 + pm],
                                     rhs=wdB_t[:], start=False, stop=True)
                    nc.vector.tensor_copy(edcols[:pm, m:m + 1], pse[:pm, :])
                nc.sync.dma_start(
                    edloc.ap().rearrange("(m p) one -> p (m one)", p=128),
                    edcols[:])

            # ---- phase 2: main slot stream (identity one-hot) ----
            with tc.tile_pool(name="gxa", bufs=6) as gxa, \
                 tc.tile_pool(name="gxb", bufs=6) as gxb, \
                 tc.tile_pool(name="oxa", bufs=3) as oxa, \
                 tc.tile_pool(name="oxb", bufs=3) as oxb, \
                 tc.tile_pool(name="sc", bufs=6) as sp, \
                 tc.tile_pool(name="selp", bufs=6) as scp, \
                 tc.tile_pool(name="wrow", bufs=2) as wrp, \
                 tc.tile_pool(name="wbc", bufs=2) as wbp, \
                 tc.tile_pool(name="rowp", bufs=4) as rowp, \
                 tc.tile_pool(name="accp", bufs=1) as accp, \
                 tc.tile_pool(name="psm", bufs=4, space="PSUM") as psm, \
                 tc.tile_pool(name="ps2", bufs=1, space="PSUM") as ps2, tc.tile_pool(name="psD", bufs=1, space="PSUM") as psD, \
                 tc.tile_pool(name="psB", bufs=1, space="PSUM") as psB, \
                 tc.tile_pool(name="evac", bufs=3) as ev:
                accs = {}

                for b0 in range(0, Cm, KB):
                    kb = min(KB, Cm - b0)
                    xa = gxa.tile([128, KB * 128], BF16, tag="xa")
                    nc.sync.dma_start(xa[:, 0:kb * 128],
                                      xmA[:, b0 * 128:(b0 + kb) * 128])
                    xb = gxb.tile([128, KB * 128], BF16, tag="xb")
                    nc.sync.dma_start(xb[:, 0:kb * 128],
                                      xmB[:, b0 * 128:(b0 + kb) * 128])
                    pss = []
                    sblk = sp.tile([128, 3 * KB], F32, tag="sblk")
                    pst = None
                    for i in range(kb):
                        q = i % 3
                        if q == 0:
                            pst = psm.tile([128, 3 * PW], F32, name="pst",
                                           tag="pst")
                        sl = pst[:, q * PW:q * PW + PW]
                        nc.tensor.matmul(out=sl,
                                         lhsT=xa[:, i * 128:(i + 1) * 128],
                                         rhs=wA_t[:], start=True, stop=False,
                                         skip_group_check=True)
                        nc.tensor.matmul(out=sl,
                                         lhsT=xb[:, i * 128:(i + 1) * 128],
                                         rhs=wB_t[:], start=False, stop=True,
                                         skip_group_check=True)
                        pss.append((pst, q))
                        nc.scalar.copy(sblk[:, i:i + 1],
                                       pst[:, q * PW + 128:q * PW + 129])
                    # scores per window segment: s = esrc + (mask + e_dst)
                    seg = b0
                    while seg < b0 + kb:
                        w = seg // R
                        seg_end = min((w + 1) * R, b0 + kb)
                        lo, hi = seg - b0, seg_end - b0
                        nc.vector.tensor_scalar(
                            sblk[:, KB + lo:KB + hi], mmask_t[:, seg:seg_end],
                            edcols[:, w:w + 1], None, op0=AO.add)
                        seg = seg_end
                    nc.vector.tensor_tensor(sblk[:, KB:KB + kb],
                                            sblk[:, KB:KB + kb],
                                            sblk[:, 0:kb], op=AO.add)
                    nc.vector.scalar_tensor_tensor(
                        sblk[:, 2 * KB:2 * KB + kb], sblk[:, KB:KB + kb], 0.2,
                        sblk[:, KB:KB + kb], op0=AO.mult, op1=AO.max)
                    # exp per window segment, accum_out -> den contribution
                    fm = sp.tile([128, KB], F32, tag="fm")
                    seg = b0
                    while seg < b0 + kb:
                        w = seg // R
                        seg_end = min((w + 1) * R, b0 + kb)
                        lo, hi = seg - b0, seg_end - b0
                        first = seg % R == 0
                        if first and w not in accs:
                            acc = accp.tile([128, PW], F32, name=f"acc_{w}",
                                            tag=f"acc_{w}")
                            accs[w] = acc
                        acc = accs[w]
                        dtmp = sp.tile([128, 1], F32, tag="dtmp")
                        nc.scalar.activation(fm[:, lo:hi],
                                             sblk[:, 2 * KB + lo:2 * KB + hi],
                                             EXP, accum_out=dtmp[:])
                        if first:
                            nc.vector.tensor_copy(acc[:, 128:129], dtmp[:])
                        else:
                            nc.vector.tensor_tensor(
                                acc[:, 128:129], acc[:, 128:129], dtmp[:],
                                op=AO.add)
                        seg = seg_end
                    for i in range(kb):
                        c = b0 + i
                        w = c // R
                        j = c - w * R
                        acc = accs[w]
                        pst, q = pss[i]
                        nc.vector.scalar_tensor_tensor(
                            acc[:, 0:128], pst[:, q * PW:q * PW + 128],
                            fm[:, i:i + 1], acc[:, 0:128], op0=AO.mult,
                            op1=(AO.bypass if j == 0 else AO.add))
                # ---- overflow stream (generic, full score matrix) ----
                if Cov > 0:
                    offs = np.zeros(NWIN + 1, np.int64)
                    offs[1:] = np.cumsum(OVC)
                    # gather X blocks for overflow chunks
                    ox_of = {}
                    for b0 in range(0, Cov, KB):
                        kb = min(KB, Cov - b0)
                        xa = oxa.tile([128, KB * 128], BF16, tag="oxa")
                        nc.sync.dma_start(xa[:, 0:kb * 128],
                                          xoA[:, b0 * 128:(b0 + kb) * 128])
                        xb = oxb.tile([128, KB * 128], BF16, tag="oxb")
                        nc.sync.dma_start(xb[:, 0:kb * 128],
                                          xoB[:, b0 * 128:(b0 + kb) * 128])
                        for i in range(kb):
                            ox_of[b0 + i] = (xa, xb, i)
                    for w in range(NWIN):
                        if OVC[w] == 0:
                            continue
                        edr = wrp.tile([1, WIN], F32, tag="edr")
                        edloc_rows = edloc.ap().rearrange(
                            "(a b) one -> a (b one)", b=WIN)
                        nc.sync.dma_start(edr[:], edloc_rows[w:w + 1, :])
                        edrb = wrp.tile([1, WIN], BF16, tag="edrb")
                        nc.vector.tensor_copy(edrb[:], edr[:])
                        edp = psB.tile([128, WIN], F32, tag="edp")
                        nc.tensor.matmul(out=edp[:], lhsT=ones_t[:],
                                         rhs=edrb[:], start=True, stop=True)
                        edw_b = wbp.tile([128, WIN], F32, tag="edw")
                        nc.vector.tensor_copy(edw_b[:], edp[:])
                        psw = ps2.tile([128, 128], F32, tag="psw")
                        psd = psD.tile([128, 1], F32, tag="psd")
                        for ci in range(int(offs[w]), int(offs[w + 1])):
                            xa, xb, i = ox_of[ci]
                            ps = psm.tile([128, 3 * PW], F32, name="pst",
                                          tag="pst")
                            nc.tensor.matmul(out=ps[:, 0:PW],
                                             lhsT=xa[:, i * 128:(i + 1) * 128],
                                             rhs=wA_t[:], start=True,
                                             stop=False,
                                             skip_group_check=True)
                            nc.tensor.matmul(out=ps[:, 0:PW],
                                             lhsT=xb[:, i * 128:(i + 1) * 128],
                                             rhs=wB_t[:], start=False,
                                             stop=True,
                                             skip_group_check=True)
                            rows = rowp.tile([128, 128], BF16, tag="rows")
                            nc.scalar.copy(rows[:], ps[:, 0:128])
                            esf = sp.tile([128, 1], F32, tag="esf")
                            nc.vector.tensor_copy(esf[:], ps[:, 128:129])
                            s0 = sp.tile([128, WIN], F32, tag="s0")
                            nc.vector.tensor_scalar(
                                s0[:], edw_b[:], esf[:, 0:1],
                                omask_t[:, ci:ci + 1], op0=AO.add, op1=AO.add)
                            s1b = sp.tile([128, WIN], F32, tag="s1b")
                            nc.vector.scalar_tensor_tensor(
                                s1b[:], s0[:], 0.2, s0[:], op0=AO.mult,
                                op1=AO.max)
                            fmm = sp.tile([128, WIN], BF16, tag="fmm")
                            nc.scalar.activation(fmm[:], s1b[:], EXP)
                            selw = scp.tile([128, 128], BF16, tag="selw")
                            nc.vector.scalar_tensor_tensor(
                                selw[:], iota_t[:], odst_t[:, ci:ci + 1],
                                fmm[:], op0=AO.is_equal, op1=AO.mult)
                            st = ci == int(offs[w])
                            sto = ci == int(offs[w + 1]) - 1
                            nc.tensor.matmul(out=psw[:], lhsT=selw[:],
                                             rhs=rows[:], start=st, stop=sto)
                            nc.tensor.matmul(out=psd[:], lhsT=selw[:],
                                             rhs=onesc_t[:], start=st,
                                             stop=sto)
                        acc = accs[w]
                        nc.vector.tensor_tensor(acc[:, 0:128], acc[:, 0:128],
                                                psw[:], op=AO.add)
                        nc.vector.tensor_tensor(acc[:, 128:129],
                                                acc[:, 128:129], psd[:],
                                                op=AO.add)

                # ---- epilogue: per window, out = elu(num/den + bias) ----
                for w in range(NWIN):
                    pw = min(128, NSH - w * 128)
                    srcv = accs[w]
                    den = ev.tile([128, 1], F32, tag="den")
                    nc.vector.tensor_scalar(den[:], srcv[:, 128:129], 1e-12,
                                            None, op0=AO.max)
                    rec = ev.tile([128, 1], F32, tag="rec")
                    nc.vector.reciprocal(rec[:], den[:])
                    o1 = ev.tile([128, 128], F32, tag="o1")
                    nc.vector.tensor_scalar(o1[:], srcv[:, 0:128], rec[:, 0:1],
                                            None, op0=AO.mult)
                    o2 = ev.tile([128, 128], F32, tag="o2")
                    nc.vector.tensor_tensor(o2[:], o1[:], obias_t[:],
                                            op=AO.add)
                    mng = ev.tile([128, 128], F32, tag="mng")
                    nc.vector.tensor_scalar(mng[:], o2[:], 0.0, None,
                                            op0=AO.min)
                    eng = ev.tile([128, 128], F32, tag="eng")
                    nc.scalar.activation(eng[:], mng[:], EXP)
                    fin = ev.tile([128, 128], F32, tag="fin")
                    nc.vector.scalar_tensor_tensor(fin[:], o2[:], 0.0, eng[:],
                                                   op0=AO.max, op1=AO.add)
                    fin2 = ev.tile([128, 128], F32, tag="fin2")
                    nc.vector.tensor_scalar(fin2[:], fin[:], 1.0, None,
                                            op0=AO.subtract)
                    nc.sync.dma_start(out[w * 128: w * 128 + pw, :],
                                      fin2[:pw, :])
    nc.compile()
    return nc


def _host_inputs(inputs):
    x = np.ascontiguousarray(np.asarray(inputs["inputs"], dtype=np.float32))
    W = np.asarray(inputs["W_seq"], dtype=np.float32)
    a_dst = np.asarray(inputs["a_dst"], dtype=np.float32)
    b_dst = np.float32(inputs["b_dst"])
    a_src = np.asarray(inputs["a_src"], dtype=np.float32)
    b_src = np.float32(inputs["b_src"])
    output_bias = np.asarray(inputs["output_bias"], dtype=np.float32)

    OVC, Cov, edge_maps = _prep_edges(inputs["edge_src"], inputs["edge_dst"])

    xb = x.astype(ml_dtypes.bfloat16)   # ship bf16, index on host
    wsrc = W @ a_src
    wdst = W @ a_dst
    wextA = np.zeros((128, PW), np.float32)
    wextA[:, 0:128] = W[0:128, :]
    wextA[:, 128] = wsrc[0:128]
    wextB = np.zeros((128, PW), np.float32)
    wextB[:, 0:128] = W[128:256, :]
    wextB[:, 128] = wsrc[128:256]
    iota = np.ascontiguousarray(
        np.tile(np.arange(128, dtype=np.float32)[None, :], (128, 1))
    ).astype(ml_dtypes.bfloat16)
    obias = np.ascontiguousarray(
        np.tile(output_bias[None, :], (128, 1))).astype(np.float32)

    in_maps = []
    for k in range(NC_):
        em = edge_maps[k]
        # bias fold: mask slab adds b_src + b_dst on real edges
        mmask = em["mmask"] + np.float32(b_src + b_dst) * (em["mmask"] == 0.0)
        omask = em["omask"] + np.float32(b_src + b_dst) * (em["omask"] == 0.0)
        xe = xb[em["mlin"]]                       # [Cm*128, 256] bf16
        xo = xb[em["olin"]]
        m = {
            "xT": np.ascontiguousarray(
                x[k * NSH:(k + 1) * NSH].T).astype(ml_dtypes.bfloat16),
            "wextA": wextA.astype(ml_dtypes.bfloat16),
            "wextB": wextB.astype(ml_dtypes.bfloat16),
            "wdA": wdst[0:128, None].astype(ml_dtypes.bfloat16),
            "wdB": wdst[128:256, None].astype(ml_dtypes.bfloat16),
            "iota": iota,
            "ones_r": np.ones((1, 128), ml_dtypes.bfloat16),
            "ones_c": np.ones((128, 1), ml_dtypes.bfloat16),
            "obias": obias,
            "xmA": np.ascontiguousarray(xe[:, 0:128].T),
            "xmB": np.ascontiguousarray(xe[:, 128:256].T),
            "mmask": np.ascontiguousarray(mmask),
            "xoA": np.ascontiguousarray(xo[:, 0:128].T),
            "xoB": np.ascontiguousarray(xo[:, 128:256].T),
            "odst": em["odst"],
            "omask": np.ascontiguousarray(omask),
        }
        in_maps.append(m)
    return OVC, Cov, in_maps


def kernel(**inputs) -> np.ndarray:
    global LAST_EXEC_NS
    OVC, Cov, in_maps = _host_inputs(inputs)
    key = (OVC, Cov, R, KB)
    if key not in _GRAPH_CACHE:
        _GRAPH_CACHE[key] = _build(OVC, Cov)
    nc = _GRAPH_CACHE[key]

    want_trace = bool(int(os.environ.get("KERNEL_TRACE", "0")))
    try:
        res = run_bass_kernel_spmd(nc, in_maps, core_ids=list(range(NC_)),
                                   trace=want_trace)
    except Exception:
        if not want_trace:
            raise
        res = run_bass_kernel_spmd(nc, in_maps, core_ids=list(range(NC_)),
                                   trace=False)
    LAST_EXEC_NS = res.exec_time_ns
    out = np.concatenate([res.results[k]["out"] for k in range(NC_)], axis=0)
    return out.astype(np.float32)


# revision 22
# speedup vs baseline: 1.0018x; 1.0018x over previous
"""GAT attention head (gnn_message_passing) on 8 TRN2 NeuronCores.

Strategy v5 (dst-sharded, gather-free recompute):
  - Per-edge h' rows are RECOMPUTED on device instead of gathered: the host
    ships X re-ordered per edge slot (X_edge, contiguous streaming reads),
    and each 128-slot chunk does two K=128 matmuls against the extended
    weight wext [256, 129] = [W | W@a_src], yielding ps = [h' (0:128) |
    e_src_raw (128)] in PSUM. This avoids all per-edge DMA descriptors
    (SWDGE desc-gen at ~8.5ns/desc and random 512-B HBM reads were the
    bottleneck of gather-based variants).
  - Slot structure: edges sharded by dst core, each dst node (w, r) owns
    partition r of R chunk-columns of its 128-dst window (identity one-hot
    => no matmul for aggregation): chunk col c = w*R + j. Per chunk, one
    fused DVE op accumulates acc[:, 0:128] += fm * ps[:, 0:128] (PSUM read,
    in-place SBUF accumulate). fm = exp(leakyrelu(e_dst + e_src + b)) with
    e_dst a per-window broadcast of the on-chip edcols column, pad masks
    (-30000) and b_src+b_dst folded into a host mask slab, and the softmax
    denominator taken from the EXP's accum_out (no ones column).
  - Leftover edges (deg > R) go to a generic overflow stream: recompute ps,
    evacuate rows to SBUF, full [128, WIN] score matrix against a PE
    broadcast of the window's e_dst row, selw = onehot*fm, and two matmuls
    (rows + ones-den) into a [128, 129] PSUM window accumulator.
  - e_dst per node comes from 2 tiny matmuls per 128-node tile against
    wd = W@a_dst (phase 1); no AllGather / collectives at all.
  - Final pass per window: out = elu(num/max(den,1e-12) + output_bias).
Output: each core writes its 6250-row slab; host concatenates.
"""

import os
import sys

for _p in ("/opt/trn_rl_repo", "/root/.axon_site/_ro/trn_rl_repo"):
    if os.path.isdir(_p) and _p not in sys.path:
        sys.path.append(_p)

import numpy as np
import ml_dtypes

import concourse.bass as bass
import concourse.mybir as mybir
import concourse.tile as tile
from concourse import bacc
from concourse.bass_utils import run_bass_kernel_spmd

NC_ = 8
N = 50000
E = 800000
IN_DIM = 256
OUT_DIM = 128
NSH = N // NC_           # 6250 nodes per core
WIN = 128                # dst window size
NWIN = (NSH + WIN - 1) // WIN   # 49
PW = 129                 # ps width: h'(128) + e_src_raw col
R = int(os.environ.get("KERNEL_R", "16"))     # slots per dst node
KB = int(os.environ.get("KERNEL_KB", "6"))    # chunks per X block
F32 = mybir.dt.float32
BF16 = mybir.dt.bfloat16

LAST_EXEC_NS = None

_GRAPH_CACHE = {}


def _prep_edges(edge_src, edge_dst):
    """Partition edges by dst core, build the fixed R-slot main grid plus a
    generic overflow stream, padded to chunk counts shared by all cores."""
    edge_src = np.asarray(edge_src).astype(np.int64)
    edge_dst = np.asarray(edge_dst).astype(np.int64)
    core = edge_dst // NSH
    grids = []
    ovfs = []
    OVC = np.zeros(NWIN, np.int64)
    for k in range(NC_):
        m = core == k
        s = edge_src[m]
        d = edge_dst[m] - k * NSH
        order = np.argsort(d, kind="stable")
        s, d = s[order], d[order]
        g = np.full((NSH, R), -1, np.int64)
        ov = [[] for _ in range(NWIN)]
        fill = np.zeros(NSH, np.int32)
        for i in range(len(s)):
            dd = d[i]
            f = fill[dd]
            if f < R:
                g[dd, f] = s[i]
                fill[dd] = f + 1
            else:
                ov[dd // WIN].append((s[i], dd - (dd // WIN) * WIN))
        grids.append(g)
        ovfs.append(ov)
        OVC = np.maximum(OVC, [(len(v) + 127) // 128 for v in ov])
    Cov = int(OVC.sum())
    ovoffs = np.zeros(NWIN + 1, np.int64)
    ovoffs[1:] = np.cumsum(OVC) * 128
    Cm = NWIN * R
    maps = []
    for k in range(NC_):
        g = grids[k]
        gfull = np.full((NWIN * WIN, R), -1, np.int64)
        gfull[:NSH] = g
        arr = gfull.reshape(NWIN, WIN, R).transpose(0, 2, 1)  # [w, j, p]
        mlin = arr.reshape(-1)                 # i = c*128 + p
        mmask = np.where(mlin >= 0, 0.0, -30000.0).astype(np.float32)
        mlin = np.where(mlin >= 0, mlin, 0)

        ov = ovfs[k]
        olin = np.zeros(max(Cov, 1) * 128, np.int64)
        odst = np.zeros(max(Cov, 1) * 128, np.float32)
        omask = np.full(max(Cov, 1) * 128, -30000.0, np.float32)
        for wv in range(NWIN):
            o = ovoffs[wv]
            for i, (src, dr) in enumerate(ov[wv]):
                olin[o + i] = src
                odst[o + i] = dr
                omask[o + i] = 0.0
        Cx = max(Cov, 1)
        maps.append({
            "mlin": mlin, "odstl": odst, "olin": olin,
            "mmask": np.ascontiguousarray(mmask.reshape(Cm, 128).T),
            "odst": np.ascontiguousarray(odst.reshape(Cx, 128).T),
            "omask": np.ascontiguousarray(omask.reshape(Cx, 128).T),
        })
    return tuple(OVC.tolist()), Cov, maps


def _build(OVC, Cov):
    Cm = NWIN * R
    nc = bacc.Bacc("TRN2", target_bir_lowering=False, debug=False,
                   enable_asserts=True, num_devices=NC_)
    xT = nc.dram_tensor("xT", [IN_DIM, NSH], BF16, kind="ExternalInput").ap()
    # wexts: [128, 129] each half: [W rows | (W@a_src) col]
    wextA = nc.dram_tensor("wextA", [128, PW], BF16, kind="ExternalInput").ap()
    wextB = nc.dram_tensor("wextB", [128, PW], BF16, kind="ExternalInput").ap()
    wdA = nc.dram_tensor("wdA", [128, 1], BF16, kind="ExternalInput").ap()
    wdB = nc.dram_tensor("wdB", [128, 1], BF16, kind="ExternalInput").ap()
    iota = nc.dram_tensor("iota", [128, 128], BF16, kind="ExternalInput").ap()
    ones_r = nc.dram_tensor("ones_r", [1, 128], BF16, kind="ExternalInput").ap()
    ones_c = nc.dram_tensor("ones_c", [128, 1], BF16, kind="ExternalInput").ap()
    obias = nc.dram_tensor("obias", [128, 128], F32, kind="ExternalInput").ap()
    xmA = nc.dram_tensor("xmA", [128, Cm * 128], BF16, kind="ExternalInput").ap()
    xmB = nc.dram_tensor("xmB", [128, Cm * 128], BF16, kind="ExternalInput").ap()
    mmask = nc.dram_tensor("mmask", [128, Cm], F32, kind="ExternalInput").ap()
    Cx = max(Cov, 1)
    xoA = nc.dram_tensor("xoA", [128, Cx * 128], BF16, kind="ExternalInput").ap()
    xoB = nc.dram_tensor("xoB", [128, Cx * 128], BF16, kind="ExternalInput").ap()
    odst = nc.dram_tensor("odst", [128, Cx], F32, kind="ExternalInput").ap()
    omask = nc.dram_tensor("omask", [128, Cx], F32, kind="ExternalInput").ap()
    out = nc.dram_tensor("out", [NSH, OUT_DIM], F32, kind="ExternalOutput").ap()

    edloc = nc.dram_tensor("edloc", [NWIN * WIN, 1], F32)

    EXP = mybir.ActivationFunctionType.Exp
    AO = mybir.AluOpType

    with tile.TileContext(nc) as tc:
        with tc.tile_pool(name="const", bufs=1) as constp, \
             tc.tile_pool(name="idx", bufs=1) as idxp:
            wA_t = constp.tile([128, PW], BF16)
            nc.sync.dma_start(wA_t[:], wextA[:, :])
            wB_t = constp.tile([128, PW], BF16)
            nc.sync.dma_start(wB_t[:], wextB[:, :])
            wdA_t = constp.tile([128, 1], BF16)
            nc.sync.dma_start(wdA_t[:], wdA[:, :])
            wdB_t = constp.tile([128, 1], BF16)
            nc.sync.dma_start(wdB_t[:], wdB[:, :])
            iota_t = constp.tile([128, 128], BF16)
            nc.sync.dma_start(iota_t[:], iota[:, :])
            ones_t = constp.tile([1, 128], BF16)
            nc.sync.dma_start(ones_t[:], ones_r[:, :])
            onesc_t = constp.tile([128, 1], BF16)
            nc.sync.dma_start(onesc_t[:], ones_c[:, :])
            obias_t = constp.tile([128, 128], F32)
            nc.sync.dma_start(obias_t[:], obias[:, :])
            edcols = constp.tile([128, NWIN], F32)
            mmask_t = idxp.tile([128, Cm], F32)
            nc.sync.dma_start(mmask_t[:], mmask[:, :])
            odst_t = idxp.tile([128, Cx], F32)
            nc.sync.dma_start(odst_t[:], odst[:, :])
            omask_t = idxp.tile([128, Cx], F32)
            nc.sync.dma_start(omask_t[:], omask[:, :])

            # ---- phase 1: per-node e_dst (edcols + edloc) ----
            with tc.tile_pool(name="p1x", bufs=1) as p1x, \
                 tc.tile_pool(name="ps1", bufs=4, space="PSUM") as ps1:
                xt = p1x.tile([128, 2 * NSH], BF16)
                nc.sync.dma_start(xt[:, 0:NSH], xT[0:128, :])
                nc.sync.dma_start(xt[:, NSH:2 * NSH], xT[128:256, :])
                nc.vector.memset(edcols[:], 0.0)
                for m in range(NWIN):
                    pm = min(128, NSH - m * 128)
                    pse = ps1.tile([128, 1], F32, tag="pse")
                    nc.tensor.matmul(out=pse[:pm, :],
                                     lhsT=xt[:, m * 128: m * 128 + pm],
                                     rhs=wdA_t[:], start=True, stop=False)
                    nc.tensor.matmul(out=pse[:pm, :],
                                     lhsT=xt[:, NSH + m * 128: NSH + m * 128 + pm],
                                     rhs=wdB_t[:], start=False, stop=True)
                    nc.vector.tensor_copy(edcols[:pm, m:m + 1], pse[:pm, :])
                nc.sync.dma_start(
                    edloc.ap().rearrange("(m p) one -> p (m one)", p=128),
                    edcols[:])

            # ---- phase 2: main slot stream (identity one-hot) ----
            with tc.tile_pool(name="gxa", bufs=6) as gxa, \
                 tc.tile_pool(name="gxb", bufs=6) as gxb, \
                 tc.tile_pool(name="oxa", bufs=3) as oxa, \
                 tc.tile_pool(name="oxb", bufs=3) as oxb, \
                 tc.tile_pool(name="sc", bufs=6) as sp, \
                 tc.tile_pool(name="selp", bufs=6) as scp, \
                 tc.tile_pool(name="wrow", bufs=2) as wrp, \
                 tc.tile_pool(name="wbc", bufs=2) as wbp, \
                 tc.tile_pool(name="rowp", bufs=4) as rowp, \
                 tc.tile_pool(name="accp", bufs=1) as accp, \
                 tc.tile_pool(name="psm", bufs=4, space="PSUM") as psm, \
                 tc.tile_pool(name="ps2", bufs=1, space="PSUM") as ps2, tc.tile_pool(name="psD", bufs=1, space="PSUM") as psD, \
                 tc.tile_pool(name="psB", bufs=1, space="PSUM") as psB, \
                 tc.tile_pool(name="evac", bufs=3) as ev:
                accs = {}

                for b0 in range(0, Cm, KB):
                    kb = min(KB, Cm - b0)
                    xa = gxa.tile([128, KB * 128], BF16, tag="xa")
                    nc.sync.dma_start(xa[:, 0:kb * 128],
                                      xmA[:, b0 * 128:(b0 + kb) * 128])
                    xb = gxb.tile([128, KB * 128], BF16, tag="xb")
                    nc.sync.dma_start(xb[:, 0:kb * 128],
                                      xmB[:, b0 * 128:(b0 + kb) * 128])
                    pss = []
                    sblk = sp.tile([128, 3 * KB], F32, tag="sblk")
                    pst = None
                    for i in range(kb):
                        q = i % 3
                        if q == 0:
                            pst = psm.tile([128, 3 * PW], F32, name="pst",
                                           tag="pst")
                        sl = pst[:, q * PW:q * PW + PW]
                        nc.tensor.matmul(out=sl,
                                         lhsT=xa[:, i * 128:(i + 1) * 128],
                                         rhs=wA_t[:], start=True, stop=False,
                                         skip_group_check=True)
                        nc.tensor.matmul(out=sl,
                                         lhsT=xb[:, i * 128:(i + 1) * 128],
                                         rhs=wB_t[:], start=False, stop=True,
                                         skip_group_check=True)
                        pss.append((pst, q))
                        nc.vector.tensor_copy(sblk[:, i:i + 1],
                                              pst[:, q * PW + 128:q * PW + 129])
                    # scores per window segment: s = esrc + (mask + e_dst)
                    seg = b0
                    while seg < b0 + kb:
                        w = seg // R
                        seg_end = min((w + 1) * R, b0 + kb)
                        lo, hi = seg - b0, seg_end - b0
                        nc.vector.tensor_scalar(
                            sblk[:, KB + lo:KB + hi], mmask_t[:, seg:seg_end],
                            edcols[:, w:w + 1], None, op0=AO.add)
                        seg = seg_end
                    nc.vector.tensor_tensor(sblk[:, KB:KB + kb],
                                            sblk[:, KB:KB + kb],
                                            sblk[:, 0:kb], op=AO.add)
                    nc.vector.scalar_tensor_tensor(
                        sblk[:, 2 * KB:2 * KB + kb], sblk[:, KB:KB + kb], 0.2,
                        sblk[:, KB:KB + kb], op0=AO.mult, op1=AO.max)
                    # exp per window segment, accum_out -> den contribution
                    fm = sp.tile([128, KB], F32, tag="fm")
                    seg = b0
                    while seg < b0 + kb:
                        w = seg // R
                        seg_end = min((w + 1) * R, b0 + kb)
                        lo, hi = seg - b0, seg_end - b0
                        first = seg % R == 0
                        if first and w not in accs:
                            acc = accp.tile([128, PW], F32, name=f"acc_{w}",
                                            tag=f"acc_{w}")
                            accs[w] = acc
                        acc = accs[w]
                        dtmp = sp.tile([128, 1], F32, tag="dtmp")
                        nc.scalar.activation(fm[:, lo:hi],
                                             sblk[:, 2 * KB + lo:2 * KB + hi],
                                             EXP, accum_out=dtmp[:])
                        if first:
                            nc.vector.tensor_copy(acc[:, 128:129], dtmp[:])
                        else:
                            nc.vector.tensor_tensor(
                                acc[:, 128:129], acc[:, 128:129], dtmp[:],
                                op=AO.add)
                        seg = seg_end
                    for i in range(kb):
                        c = b0 + i
                        w = c // R
                        j = c - w * R
                        acc = accs[w]
                        pst, q = pss[i]
                        nc.vector.scalar_tensor_tensor(
                            acc[:, 0:128], pst[:, q * PW:q * PW + 128],
                            fm[:, i:i + 1], acc[:, 0:128], op0=AO.mult,
                            op1=(AO.bypass if j == 0 else AO.add))
                # ---- overflow stream (generic, full score matrix) ----
                if Cov > 0:
                    offs = np.zeros(NWIN + 1, np.int64)
                    offs[1:] = np.cumsum(OVC)
                    # gather X blocks for overflow chunks
                    ox_of = {}
                    for b0 in range(0, Cov, KB):
                        kb = min(KB, Cov - b0)
                        xa = oxa.tile([128, KB * 128], BF16, tag="oxa")
                        nc.sync.dma_start(xa[:, 0:kb * 128],
                                          xoA[:, b0 * 128:(b0 + kb) * 128])
                        xb = oxb.tile([128, KB * 128], BF16, tag="oxb")
                        nc.sync.dma_start(xb[:, 0:kb * 128],
                                          xoB[:, b0 * 128:(b0 + kb) * 128])
                        for i in range(kb):
                            ox_of[b0 + i] = (xa, xb, i)
                    for w in range(NWIN):
                        if OVC[w] == 0:
                            continue
                        edr = wrp.tile([1, WIN], F32, tag="edr")
                        edloc_rows = edloc.ap().rearrange(
                            "(a b) one -> a (b one)", b=WIN)
                        nc.sync.dma_start(edr[:], edloc_rows[w:w + 1, :])
                        edrb = wrp.tile([1, WIN], BF16, tag="edrb")
                        nc.vector.tensor_copy(edrb[:], edr[:])
                        edp = psB.tile([128, WIN], F32, tag="edp")
                        nc.tensor.matmul(out=edp[:], lhsT=ones_t[:],
                                         rhs=edrb[:], start=True, stop=True)
                        edw_b = wbp.tile([128, WIN], F32, tag="edw")
                        nc.vector.tensor_copy(edw_b[:], edp[:])
                        psw = ps2.tile([128, 128], F32, tag="psw")
                        psd = psD.tile([128, 1], F32, tag="psd")
                        for ci in range(int(offs[w]), int(offs[w + 1])):
                            xa, xb, i = ox_of[ci]
                            ps = psm.tile([128, 3 * PW], F32, name="pst",
                                          tag="pst")
                            nc.tensor.matmul(out=ps[:, 0:PW],
                                             lhsT=xa[:, i * 128:(i + 1) * 128],
                                             rhs=wA_t[:], start=True,
                                             stop=False,
                                             skip_group_check=True)
                            nc.tensor.matmul(out=ps[:, 0:PW],
                                             lhsT=xb[:, i * 128:(i + 1) * 128],
                                             rhs=wB_t[:], start=False,
                                             stop=True,
                                             skip_group_check=True)
                            rows = rowp.tile([128, 128], BF16, tag="rows")
                            nc.vector.tensor_copy(rows[:], ps[:, 0:128])
                            esf = sp.tile([128, 1], F32, tag="esf")
                            nc.vector.tensor_copy(esf[:], ps[:, 128:129])
                            s0 = sp.tile([128, WIN], F32, tag="s0")
                            nc.vector.tensor_scalar(
                                s0[:], edw_b[:], esf[:, 0:1],
                                omask_t[:, ci:ci + 1], op0=AO.add, op1=AO.add)
                            s1b = sp.tile([128, WIN], F32, tag="s1b")
                            nc.vector.scalar_tensor_tensor(
                                s1b[:], s0[:], 0.2, s0[:], op0=AO.mult,
                                op1=AO.max)
                            fmm = sp.tile([128, WIN], BF16, tag="fmm")
                            nc.scalar.activation(fmm[:], s1b[:], EXP)
                            selw = scp.tile([128, 128], BF16, tag="selw")
                            nc.vector.scalar_tensor_tensor(
                                selw[:], iota_t[:], odst_t[:, ci:ci + 1],
                                fmm[:], op0=AO.is_equal, op1=AO.mult)
                            st = ci == int(offs[w])
                            sto = ci == int(offs[w + 1]) - 1
                            nc.tensor.matmul(out=psw[:], lhsT=selw[:],
                                             rhs=rows[:], start=st, stop=sto)
                            nc.tensor.matmul(out=psd[:], lhsT=selw[:],
                                             rhs=onesc_t[:], start=st,
                                             stop=sto)
                        acc = accs[w]
                        nc.vector.tensor_tensor(acc[:, 0:128], acc[:, 0:128],
                                                psw[:], op=AO.add)
                        nc.vector.tensor_tensor(acc[:, 128:129],
                                                acc[:, 128:129], psd[:],
                                                op=AO.add)

                # ---- epilogue: per window, out = elu(num/den + bias) ----
                for w in range(NWIN):
                    pw = min(128, NSH - w * 128)
                    srcv = accs[w]
                    den = ev.tile([128, 1], F32, tag="den")
                    nc.vector.tensor_scalar(den[:], srcv[:, 128:129], 1e-12,
                                            None, op0=AO.max)
                    rec = ev.tile([128, 1], F32, tag="rec")
                    nc.vector.reciprocal(rec[:], den[:])
                    o1 = ev.tile([128, 128], F32, tag="o1")
                    nc.vector.tensor_scalar(o1[:], srcv[:, 0:128], rec[:, 0:1],
                                            None, op0=AO.mult)
                    o2 = ev.tile([128, 128], F32, tag="o2")
                    nc.vector.tensor_tensor(o2[:], o1[:], obias_t[:],
                                            op=AO.add)
                    mng = ev.tile([128, 128], F32, tag="mng")
                    nc.vector.tensor_scalar(mng[:], o2[:], 0.0, None,
                                            op0=AO.min)
                    eng = ev.tile([128, 128], F32, tag="eng")
                    nc.scalar.activation(eng[:], mng[:], EXP)
                    fin = ev.tile([128, 128], F32, tag="fin")
                    nc.vector.scalar_tensor_tensor(fin[:], o2[:], 0.0, eng[:],
                                                   op0=AO.max, op1=AO.add)
                    fin2 = ev.tile([128, 128], F32, tag="fin2")
                    nc.vector.tensor_scalar(fin2[:], fin[:], 1.0, None,
                                            op0=AO.subtract)
                    nc.sync.dma_start(out[w * 128: w * 128 + pw, :],
                                      fin2[:pw, :])
    nc.compile()
    return nc


def _host_inputs(inputs):
    x = np.ascontiguousarray(np.asarray(inputs["inputs"], dtype=np.float32))
    W = np.asarray(inputs["W_seq"], dtype=np.float32)
    a_dst = np.asarray(inputs["a_dst"], dtype=np.float32)
    b_dst = np.float32(inputs["b_dst"])
    a_src = np.asarray(inputs["a_src"], dtype=np.float32)
    b_src = np.float32(inputs["b_src"])
    output_bias = np.asarray(inputs["output_bias"], dtype=np.float32)

    OVC, Cov, edge_maps = _prep_edges(inputs["edge_src"], inputs["edge_dst"])

    xb = x.astype(ml_dtypes.bfloat16)   # ship bf16, index on host
    wsrc = W @ a_src
    wdst = W @ a_dst
    wextA = np.zeros((128, PW), np.float32)
    wextA[:, 0:128] = W[0:128, :]
    wextA[:, 128] = wsrc[0:128]
    wextB = np.zeros((128, PW), np.float32)
    wextB[:, 0:128] = W[128:256, :]
    wextB[:, 128] = wsrc[128:256]
    iota = np.ascontiguousarray(
        np.tile(np.arange(128, dtype=np.float32)[None, :], (128, 1))
    ).astype(ml_dtypes.bfloat16)
    obias = np.ascontiguousarray(
        np.tile(output_bias[None, :], (128, 1))).astype(np.float32)

    in_maps = []
    for k in range(NC_):
        em = edge_maps[k]
        # bias fold: mask slab adds b_src + b_dst on real edges
        mmask = em["mmask"] + np.float32(b_src + b_dst) * (em["mmask"] == 0.0)
        omask = em["omask"] + np.float32(b_src + b_dst) * (em["omask"] == 0.0)
        xe = xb[em["mlin"]]                       # [Cm*128, 256] bf16
        xo = xb[em["olin"]]
        m = {
            "xT": np.ascontiguousarray(
                x[k * NSH:(k + 1) * NSH].T).astype(ml_dtypes.bfloat16),
            "wextA": wextA.astype(ml_dtypes.bfloat16),
            "wextB": wextB.astype(ml_dtypes.bfloat16),
            "wdA": wdst[0:128, None].astype(ml_dtypes.bfloat16),
            "wdB": wdst[128:256, None].astype(ml_dtypes.bfloat16),
            "iota": iota,
            "ones_r": np.ones((1, 128), ml_dtypes.bfloat16),
            "ones_c": np.ones((128, 1), ml_dtypes.bfloat16),
            "obias": obias,
            "xmA": np.ascontiguousarray(xe[:, 0:128].T),
            "xmB": np.ascontiguousarray(xe[:, 128:256].T),
            "mmask": np.ascontiguousarray(mmask),
            "xoA": np.ascontiguousarray(xo[:, 0:128].T),
            "xoB": np.ascontiguousarray(xo[:, 128:256].T),
            "odst": em["odst"],
            "omask": np.ascontiguousarray(omask),
        }
        in_maps.append(m)
    return OVC, Cov, in_maps


def kernel(**inputs) -> np.ndarray:
    global LAST_EXEC_NS
    OVC, Cov, in_maps = _host_inputs(inputs)
    key = (OVC, Cov, R, KB)
    if key not in _GRAPH_CACHE:
        _GRAPH_CACHE[key] = _build(OVC, Cov)
    nc = _GRAPH_CACHE[key]

    want_trace = bool(int(os.environ.get("KERNEL_TRACE", "0")))
    try:
        res = run_bass_kernel_spmd(nc, in_maps, core_ids=list(range(NC_)),
                                   trace=want_trace)
    except Exception:
        if not want_trace:
            raise
        res = run_bass_kernel_spmd(nc, in_maps, core_ids=list(range(NC_)),
                                   trace=False)
    LAST_EXEC_NS = res.exec_time_ns
    out = np.concatenate([res.results[k]["out"] for k in range(NC_)], axis=0)
    return out.astype(np.float32)
